# revision 42
# baseline (speedup 1.0000x reference)
"""Memory-efficient multi-head attention on 8 Trainium2 NeuronCores.

Sharding: tensor-parallel over heads (4 head-groups) x data-parallel over
batch (2) = 8 cores. Core c handles head group g = c % 4 (heads 4g..4g+3,
feature slice 512) of batch b = c // 4. Each core computes its Q/K/V
projections from sliced weights, attention for its 4 heads, and a partial
out-projection y_c = ao_c @ Wo[:, gs].T; the host sums the 4 partials per
batch and adds the output bias.

All matmuls run in fp16 (same PE rate as bf16, ~3e-4 end-to-end error,
half the DMA/SBUF of fp32, and fast-weight-load eligible so LDWEIGHTS
hides behind the matmul stream — fp32r weights load serially at 4B and
cost ~100us across the kernel). fp8 was measured and rejected: attention
output is itself an average over ~2k keys, so elementwise quantization
noise in V or exp(scores) does NOT average away (2.7% each), and the
fp8 Q/K path measures 2.3e-2 against the 2e-2 tolerance.

Softmax: scores/exp per 1024-wide PSUM tile; row-sums via ones-vector
matmuls accumulated alongside attn@V; denominator broadcast across
partitions with a K=1 matmul, inverted with reciprocal_approx_fast
(exact reciprocal on [1,512] costs 3.3us vs ~0.7us here), and a single
VectorE multiply writes the normalized attention output.
"""

import sys

if "/opt/trn_rl_repo" not in sys.path:
    sys.path.insert(0, "/opt/trn_rl_repo")

from contextlib import ExitStack

import numpy as np

import concourse.bacc as bacc
import concourse.mybir as mybir
import concourse.tile as tile
from concourse.bass_utils import run_bass_kernel_spmd

B, S, D, H = 2, 2048, 2048, 16
HD = 128               # head dim
G = 4                  # head groups (tensor-parallel degree)
HPG = H // G           # heads per group = 4
FC = HPG * HD          # per-core feature slice = 512
KC = D // 128          # contraction chunks = 16
SB = 4                 # seq blocks (512 wide)
QB = 4                 # q blocks (512 wide)
ST = S // 128          # seq tiles = 16
SCALE = float(HD) ** -0.5
# fine-grained contraction-chunk sweep: each small DMA matches the matmul
# consumption rate of one sweep pass, so the projections track the
# weight/x transfers without burst stalls; 1-chunk head starts earliest
_CHUNKS = [slice(0, 1), slice(1, 2)] + \
    [slice(2 * i, 2 * i + 2) for i in range(1, 8)]

F32 = mybir.dt.float32
F16 = mybir.dt.float16

PROFILE = False        # set by test.py to collect an NTFF trace
LAST = {}              # exec_time_ns etc. stashed here when PROFILE

_cache = {}


def _build(masked: bool):
    nc = bacc.Bacc("TRN2", target_bir_lowering=False)

    xb = nc.dram_tensor("xb", (D, S), F16, kind="ExternalInput")
    wqh = nc.dram_tensor("wqh", (D, FC), F16, kind="ExternalInput")
    wkh = nc.dram_tensor("wkh", (D, FC), F16, kind="ExternalInput")
    wvh = nc.dram_tensor("wvh", (D, FC), F16, kind="ExternalInput")
    woh = nc.dram_tensor("woh", (FC, D), F16, kind="ExternalInput")
    bq2 = nc.dram_tensor("bq2", (128, HPG), F32, kind="ExternalInput")
    bk2 = nc.dram_tensor("bk2", (128, HPG), F32, kind="ExternalInput")
    bvb = nc.dram_tensor("bvb", (128, FC), F32, kind="ExternalInput")
    onesm = nc.dram_tensor("onesm", (128, 128), F16, kind="ExternalInput")
    mT = None
    if masked:
        mT = nc.dram_tensor("mT", (S, S), F32, kind="ExternalInput")
    y = nc.dram_tensor("y", (S, D), F16, kind="ExternalOutput")

    xb_v = xb[:].rearrange("(c p) s -> p c s", p=128)
    wq_v = wqh[:].rearrange("(c p) f -> p c f", p=128)
    wk_v = wkh[:].rearrange("(c p) f -> p c f", p=128)
    wv_v = wvh[:].rearrange("(c p) f -> p c f", p=128)
    wo_v = woh[:].rearrange("(c p) f -> p c f", p=128)
    mT_v = mT[:].rearrange("(c p) q -> p c q", p=128) if masked else None

    EXP = mybir.ActivationFunctionType.Exp
    IDN = mybir.ActivationFunctionType.Identity

    with tile.TileContext(nc) as tc, ExitStack() as top:
        const = top.enter_context(tc.tile_pool(name="const", bufs=1))
        store = top.enter_context(tc.tile_pool(name="store", bufs=1))
        attp = top.enter_context(tc.tile_pool(name="attp", bufs=1))

        t_bq = const.tile([128, HPG], F32, tag="bq")
        t_bk = const.tile([128, HPG], F32, tag="bk")
        t_bvb = const.tile([128, FC], F32, tag="bvb")
        t_ones = const.tile([128, 128], F16, tag="ones")
        nc.gpsimd.dma_start(t_bq[:], bq2[:])
        nc.gpsimd.dma_start(t_bk[:], bk2[:])
        nc.gpsimd.dma_start(t_bvb[:], bvb[:])
        nc.gpsimd.dma_start(t_ones[:], onesm[:])

        QT = [store.tile([128, S], F16, tag=f"qt{h}", name=f"qt{h}")
              for h in range(HPG)]
        KT = [store.tile([128, S], F16, tag=f"kt{h}", name=f"kt{h}")
              for h in range(HPG)]
        V = [store.tile([128, FC], F16, tag=f"v{kt}", name=f"v{kt}")
             for kt in range(ST)]
        AO = [store.tile([128, S], F16, tag=f"ao{h}", name=f"ao{h}")
              for h in range(HPG)]
        WoS = [store.tile([128, D], F16, tag=f"wo{fc}", name=f"wo{fc}")
               for fc in range(HPG)]

        with tc.tile_pool(name="ps", bufs=1, space="PSUM") as psp:
            # attention iteration space + scores helper, defined early so
            # phase 1 can prefetch the first two score tiles (hides the
            # first exp's latency behind the last V-projection matmuls)
            iters = [(h, qb, ktp)
                     for h in range(HPG)
                     for qb in range(QB)
                     for ktp in range(8)]

            def scores(h, qb, ktp):
                t = psp.tile([128, 1024], F32, tag="s", bufs=2,
                             name="ps_s")
                qsl = slice(qb * 512, (qb + 1) * 512)
                for half in range(2):
                    kt = 2 * ktp + half
                    nc.tensor.matmul(
                        t[:, half * 512 : (half + 1) * 512],
                        KT[h][:, kt * 128 : (kt + 1) * 128],
                        QT[h][:, qsl],
                        start=True,
                        stop=True,
                    )
                return t

            pre = []

            # ---- phase 1: Q/K/V projections --------------------------
            with tc.tile_pool(name="wp", bufs=1) as wp, \
                 tc.tile_pool(name="xp", bufs=1) as xp:
                t_wq = wp.tile([128, KC, FC], F16, tag="wq")
                t_wk = wp.tile([128, KC, FC], F16, tag="wk")
                t_wv = wp.tile([128, KC, FC], F16, tag="wv")
                # chunked so partial arrival unlocks the chunk-progressive
                # matmul sweeps (a single dma_start only signals when the
                # whole transfer lands). The first two x chunks ride the
                # sync queue, which starts ~2us before scalar's preamble
                # ends, so the very first matmuls aren't gated on scalar.
                txb0 = xp.tile([128, KC, 512], F16, tag="xb", bufs=2)
                for ci, csl in enumerate(_CHUNKS):
                    nc.sync.dma_start(t_wq[:, csl, :], wq_v[:, csl, :])
                    if ci < 2:
                        nc.sync.dma_start(txb0[:, csl, :],
                                          xb_v[:, csl, 0:512])
                for csl in _CHUNKS:
                    nc.sync.dma_start(t_wk[:, csl, :], wk_v[:, csl, :])
                for csl in _CHUNKS:
                    nc.sync.dma_start(t_wv[:, csl, :], wv_v[:, csl, :])
                for fc in range(HPG):
                    nc.sync.dma_start(WoS[fc][:], wo_v[:, fc, :])

                for sb in range(SB):
                    ssl = slice(sb * 512, (sb + 1) * 512)
                    if sb == 0:
                        txb = txb0
                        for csl in _CHUNKS[2:]:
                            nc.scalar.dma_start(txb[:, csl, :],
                                                xb_v[:, csl, ssl])
                    else:
                        txb = xp.tile([128, KC, 512], F16, tag="xb",
                                      bufs=2)
                        nc.scalar.dma_start(txb[:], xb_v[:, :, ssl])
                    # chunk-progressive sweep over 4 concurrent PSUM groups
                    # so matmul consumption tracks the weight/x DMA arrival
                    # instead of each group demanding all 16 chunks at once
                    for wt, bias_t, dst in ((t_wq, t_bq, QT),
                                            (t_wk, t_bk, KT)):
                        pss = [psp.tile([128, 512], F32, tag="p1", bufs=4,
                                        name=f"psqk{mt}")
                               for mt in range(HPG)]
                        for csl in _CHUNKS:
                            for mt in range(HPG):
                                for kc in range(csl.start, csl.stop):
                                    nc.tensor.matmul(
                                        pss[mt][:],
                                        wt[:, kc,
                                           mt * 128 : (mt + 1) * 128],
                                        txb[:, kc, :],
                                        start=(kc == 0),
                                        stop=(kc == KC - 1),
                                    )
                        for mt in range(HPG):
                            nc.scalar.activation(
                                dst[mt][:, ssl], pss[mt][:], IDN,
                                bias=bias_t[:, mt : mt + 1], scale=1.0,
                            )
                    if sb == SB - 1:
                        pre.append(scores(*iters[0]))
                        pre.append(scores(*iters[1]))
                    psv = [psp.tile([128, 512], F32, tag="p1", bufs=4,
                                    name=f"psv{j}")
                           for j in range(4)]
                    for csl in _CHUNKS:
                        for j in range(4):
                            for kc in range(csl.start, csl.stop):
                                nc.tensor.matmul(
                                    psv[j][:],
                                    txb[:, kc, j * 128 : (j + 1) * 128],
                                    t_wv[:, kc, :],
                                    start=(kc == 0),
                                    stop=(kc == KC - 1),
                                )
                    for j in range(4):
                        kt = sb * 4 + j
                        nc.vector.tensor_add(V[kt][:], psv[j][:], t_bvb[:])

            # ---- phase 2: attention ----------------------------------
            # one flat software pipeline over (head, qb, kt-pair) so the
            # scores->exp->attn@V chain keeps one-iteration lookahead
            # across (head, qb) boundaries too
            avsm = {}

            def flush_sums(pending):
                for sm, e, esl, st, sp in pending:
                    nc.tensor.matmul(sm[:], t_ones[:], e[:, esl],
                                     start=st, stop=sp)

            def norm(done_hq):
                # normalize: ps_sm rows are already the broadcast sums
                dh, dqsl, dav, dsm = done_hq
                bcr = attp.tile([128, 512], F32, tag="bcr", bufs=2)
                nc.vector.reciprocal_approx_fast(bcr[:], dsm[:])
                nc.vector.tensor_mul(AO[dh][:, dqsl], bcr[:], dav[:])

            cur = pre[0]
            pending = []   # sum matmuls lagged one iteration: they read the
            done_hq = None  # previous exp tile, so they fill the PE while
            for i, (h, qb, ktp) in enumerate(iters):  # ACT finishes exp(i)
                qsl = slice(qb * 512, (qb + 1) * 512)
                if i == 0:
                    nxt = pre[1]
                elif i + 1 < len(iters):
                    nxt = scores(*iters[i + 1])
                else:
                    nxt = None
                if ktp == 0:
                    ps_av = psp.tile([128, 512], F32, tag="p1", bufs=4,
                                     name="av")
                    # [128,128] ones stationary: every matmul keeps the
                    # same tile shape (shape switches cost ~95ns of
                    # exposed LDWEIGHTS each), and the output arrives
                    # already broadcast across partitions.
                    ps_sm = psp.tile([128, 512], F32, tag="p1", bufs=4,
                                     name="sm")
                    avsm[(h, qb)] = (ps_av, ps_sm)
                ps_av, ps_sm = avsm[(h, qb)]
                et = attp.tile([128, 1024], F16, tag="et", bufs=6)
                nc.scalar.activation(et[:], cur[:], EXP, scale=SCALE)
                if masked:
                    mtile = attp.tile([128, 2, 512], F32,
                                      tag="mtile", bufs=3)
                    nc.sync.dma_start(
                        mtile[:], mT_v[:, 2 * ktp : 2 * ktp + 2, qsl]
                    )
                    nc.vector.tensor_mul(
                        et[:], et[:],
                        mtile[:].rearrange("p c q -> p (c q)"),
                    )
                flush_sums(pending)
                pending = []
                if done_hq is not None:
                    norm(done_hq)
                    done_hq = None
                for half in range(2):
                    kt = 2 * ktp + half
                    esl = slice(half * 512, (half + 1) * 512)
                    nc.tensor.matmul(
                        ps_av[:],
                        V[kt][:, h * 128 : (h + 1) * 128],
                        et[:, esl],
                        start=(kt == 0),
                        stop=(kt == ST - 1),
                    )
                    pending.append((ps_sm, et, esl, kt == 0, kt == ST - 1))
                cur = nxt
                if ktp == 7:
                    done_hq = (h, qsl, ps_av, ps_sm)
            flush_sums(pending)
            norm(done_hq)

        # ---- phase 3: out-projection ------------------------------
        with tc.tile_pool(name="p3s", bufs=1) as p3s, \
             tc.tile_pool(name="p3ps", bufs=1, space="PSUM") as p3ps:
            for st in range(ST):
                stsl = slice(st * 128, (st + 1) * 128)
                psy = p3ps.tile([128, D], F32, tag="psy", bufs=2)
                for fc in range(HPG):
                    for dcb in range(4):
                        nc.tensor.matmul(
                            psy[:, dcb * 512 : (dcb + 1) * 512],
                            AO[fc][:, stsl],
                            WoS[fc][:, dcb * 512 : (dcb + 1) * 512],
                            start=(fc == 0),
                            stop=(fc == HPG - 1),
                        )
                yt = p3s.tile([128, D], F16, tag="yt", bufs=3)
                if st < ST - 1:
                    nc.scalar.copy(yt[:, 0:1024], psy[:, 0:1024])
                    nc.sync.dma_start(y[stsl, 0:1024], yt[:, 0:1024])
                    nc.vector.tensor_copy(yt[:, 1024:2048],
                                          psy[:, 1024:2048])
                    nc.sync.dma_start(y[stsl, 1024:2048], yt[:, 1024:2048])
                else:
                    # quarter the last tile's evacuation so the final store
                    # chain off the critical path is as short as possible
                    for q4 in range(4):
                        cs = slice(q4 * 512, (q4 + 1) * 512)
                        eng = nc.scalar.copy if q4 % 2 == 0 else \
                            nc.vector.tensor_copy
                        eng(yt[:, cs], psy[:, cs])
                        nc.sync.dma_start(y[stsl, cs], yt[:, cs])

    nc.finalize()
    return nc


def _in_maps(x, mask, Wq, bq, Wk, bk, Wv, bv, Wo, bo, masked):
    om = np.ones((128, 128), np.float16)
    per_batch = [
        np.ascontiguousarray(x[b].T).astype(np.float16) for b in range(B)
    ]
    mTb = None
    if masked:
        mTb = [
            np.ascontiguousarray((mask[b, 0] != 0).T.astype(np.float32))
            for b in range(B)
        ]
    in_maps = []
    for c in range(8):
        g, b = c % G, c // G
        gs = slice(g * FC, (g + 1) * FC)
        m = {
            "xb": per_batch[b],
            "wqh": np.ascontiguousarray(Wq[gs].T).astype(np.float16),
            "wkh": np.ascontiguousarray(Wk[gs].T).astype(np.float16),
            "wvh": np.ascontiguousarray(Wv[gs].T).astype(np.float16),
            "woh": np.ascontiguousarray(Wo[:, gs].T).astype(np.float16),
            "bq2": np.ascontiguousarray(bq[gs].reshape(HPG, 128).T),
            "bk2": np.ascontiguousarray(bk[gs].reshape(HPG, 128).T),
            "bvb": np.tile(bv[gs][None, :], (128, 1)).astype(np.float32),
            "onesm": om,
        }
        if masked:
            m["mT"] = mTb[b]
        in_maps.append(m)
    return in_maps


def kernel(x, mask, Wq, bq, Wk, bk, Wv, bv, Wo, bo):
    x = np.asarray(x, dtype=np.float32)
    mask = np.asarray(mask)
    Wq, bq = np.asarray(Wq, np.float32), np.asarray(bq, np.float32)
    Wk, bk = np.asarray(Wk, np.float32), np.asarray(bk, np.float32)
    Wv, bv = np.asarray(Wv, np.float32), np.asarray(bv, np.float32)
    Wo, bo = np.asarray(Wo, np.float32), np.asarray(bo, np.float32)

    masked = bool((mask == 0).any())
    if masked not in _cache:
        _cache[masked] = _build(masked)
    nc = _cache[masked]

    in_maps = _in_maps(x, mask, Wq, bq, Wk, bk, Wv, bv, Wo, bo, masked)

    res = run_bass_kernel_spmd(
        nc, in_maps, core_ids=list(range(8)), trace=PROFILE
    )
    if PROFILE:
        LAST["exec_time_ns"] = res.exec_time_ns
        LAST["profile_json"] = res.profile_json
        LAST["trace"] = res.instructions_and_trace

    out = np.empty((B, S, D), np.float32)
    for b in range(B):
        acc = res.results[4 * b]["y"].astype(np.float64)
        for g in range(1, G):
            acc += res.results[4 * b + g]["y"].astype(np.float64)
        out[b] = (acc + bo).astype(np.float32)
    return out
